# revision 5
# baseline (speedup 1.0000x reference)
"""3D Haar DWT (depth-1) Trainium2 kernel.

Full inputs: x [4, 4, 64, 256, 256] f32 + six banded Haar matrices
(hardcoded math: every output element is +-2^-1.5 times a +-sum of a
2x2x2 block). Returns the 8 subbands (LLL, LLH, LHL, LHH, HLL, HLH,
HHL, HHH), each [4, 4, 32, 128, 128] f32.

Sharding: data-parallel over N*C = 16 sample-channels, 2 per core on
8 cores. Per-core compute is a 3-stage butterfly:
  H stage: pairs of rows      -> TensorE matmul (fp32, weights +-2^-1.5)
                                 for 3 of every 4 d-pairs; DVE adds on
                                 DMA-deinterleaved rows for the 4th.
  W stage: pairs of columns   -> DVE stride-2 tensor_add/sub
  D stage: pairs of d-slices  -> DVE tensor_add/sub
ScalarE evacuates PSUM (H results) and pre-scales the DVE-branch
inputs by 2^-1.5, so everything stays fp32-exact.
"""
import sys

sys.path.insert(0, "/opt/trn_rl_repo")

import numpy as np

N, C, D, H, W = 4, 4, 64, 256, 256
NCORES = 8
G_PER_CORE = (N * C) // NCORES        # 2
KP = D // 2                           # 32 d-pairs per g
S3 = float(2.0 ** -1.5)

# schedule tunables
PE_PATTERN = (True, True, True, False)  # per 4-pair batch: 3 PE + 1 DVE
KB = 8                                  # k-slices per output staging block
IN_BUFS = 8
EV_BUFS = 3
WT_BUFS = 2
OS_BUFS = 2
PSUM_BUFS = 3

_CACHE = {}


def _build_filter_lhst():
    """F = S3 * [[pairwise low]; [pairwise high]] (256x256); return F^T
    as [2 (K tile), 2 (M tile), 128, 128] fp32 for the matmul lhsT."""
    F = np.zeros((H, H), dtype=np.float32)
    s = np.float32(S3)
    for i in range(H // 2):
        F[i, 2 * i] = s
        F[i, 2 * i + 1] = s
        F[H // 2 + i, 2 * i] = s
        F[H // 2 + i, 2 * i + 1] = -s
    FT = F.T.copy()  # [h, h']
    out = np.zeros((2, 2, 128, 128), dtype=np.float32)
    for k in range(2):
        for m in range(2):
            out[k, m] = FT[k * 128:(k + 1) * 128, m * 128:(m + 1) * 128]
    return out


def _build_nc():
    import concourse.bass as bass
    import concourse.tile as tile
    from concourse import bacc, mybir

    f32 = mybir.dt.float32
    nc = bacc.Bacc(None)
    x_d = nc.declare_dram_parameter("x", [G_PER_CORE, D, H, W], f32,
                                    isOutput=False)
    ft_d = nc.declare_dram_parameter("ft", [2, 2, 128, 128], f32,
                                     isOutput=False)
    # h'-major layout: per (s, g, partition=h') a k-block of 8 is one
    # contiguous 4 KiB run in DRAM (host transposes k and h' back)
    o_d = nc.declare_dram_parameter("out", [8, G_PER_CORE, 128, KP, 128],
                                    f32, isOutput=True)

    with tile.TileContext(nc) as tc:
        with (
            tc.tile_pool(name="cst", bufs=1) as cst,
            tc.tile_pool(name="inp", bufs=IN_BUFS) as inp,
            tc.tile_pool(name="ev", bufs=EV_BUFS) as evp,
            tc.tile_pool(name="wt", bufs=WT_BUFS) as wtp,
            tc.tile_pool(name="os", bufs=OS_BUFS) as osp,
            tc.tile_pool(name="ps", bufs=PSUM_BUFS, space="PSUM") as psp,
        ):
            ft = cst.tile([128, 512], f32, tag="ft")
            nc.sync.dma_start(
                ft.rearrange("p (k m c) -> p k m c", k=2, m=2),
                ft_d.rearrange("k m p c -> p k m c"))

            def ft_block(kt, m):
                return ft[:, (kt * 2 + m) * 128:(kt * 2 + m + 1) * 128]

            for g in range(G_PER_CORE):
                for kb in range(KP // KB):
                    os_t = osp.tile([128, 8 * KB * 128], f32, tag="os")
                    for half in range(KB // 4):
                        wt_t = wtp.tile([128, 4 * 1024], f32, tag="wt")
                        for j in range(4):
                            k = kb * KB + half * 4 + j
                            use_pe = PE_PATTERN[j]
                            base = j * 1024
                            if use_pe:
                                # --- H stage on TensorE ---
                                # xa: h 0:128 of both slices, xb: h 128:256
                                xab = inp.tile([128, 1024], f32,
                                               tag="xin")
                                src = x_d[g, 2 * k:2 * k + 2]
                                nc.sync.dma_start(
                                    xab[:, 0:512].rearrange(
                                        "p (s w) -> p s w", s=2),
                                    src[:, 0:128, :].rearrange(
                                        "s h w -> h s w"))
                                nc.sync.dma_start(
                                    xab[:, 512:1024].rearrange(
                                        "p (s w) -> p s w", s=2),
                                    src[:, 128:256, :].rearrange(
                                        "s h w -> h s w"))
                                xa = xab[:, 0:512]
                                xb = xab[:, 512:1024]
                                pt = psp.tile([128, 1024], f32, tag="ps")
                                for m in range(2):
                                    nc.tensor.matmul(
                                        pt[:, m * 512:(m + 1) * 512],
                                        ft_block(0, m), xa, start=True,
                                        stop=False)
                                    nc.tensor.matmul(
                                        pt[:, m * 512:(m + 1) * 512],
                                        ft_block(1, m), xb, start=False,
                                        stop=True)
                                # ScalarE evacuation (already scaled by S3
                                # via the filter weights)
                                evt = evp.tile([128, 1024], f32, tag="ev")
                                nc.scalar.activation(
                                    evt[:], pt[:],
                                    mybir.ActivationFunctionType.Copy)
                                # --- W stage on DVE (stride 2) ---
                                nc.vector.tensor_add(
                                    wt_t[:, base:base + 512],
                                    evt[:, 0::2], evt[:, 1::2])
                                nc.vector.tensor_sub(
                                    wt_t[:, base + 512:base + 1024],
                                    evt[:, 0::2], evt[:, 1::2])
                            else:
                                # --- H stage on DVE (deinterleaved rows) ---
                                eo = inp.tile([128, 1024], f32,
                                              tag="xin")
                                src = x_d[g, 2 * k:2 * k + 2]
                                nc.sync.dma_start(
                                    eo[:, 0:512].rearrange(
                                        "p (s w) -> p s w", s=2),
                                    src[:, 0::2, :].rearrange(
                                        "s h w -> h s w"))
                                nc.sync.dma_start(
                                    eo[:, 512:1024].rearrange(
                                        "p (s w) -> p s w", s=2),
                                    src[:, 1::2, :].rearrange(
                                        "s h w -> h s w"))
                                # ScalarE applies the S3 scale in place
                                nc.scalar.activation(
                                    eo[:], eo[:],
                                    mybir.ActivationFunctionType.Copy,
                                    bias=0.0, scale=S3)
                                et = eo[:, 0:512]
                                ot = eo[:, 512:1024]
                                pl = evp.tile([128, 1024], f32, tag="ev")
                                # pl = {A_lo | A_hi}, each {s0 w | s1 w}
                                nc.vector.tensor_add(pl[:, 0:512], et, ot)
                                nc.vector.tensor_sub(pl[:, 512:1024], et, ot)
                                # --- W stage ---
                                nc.vector.tensor_add(
                                    wt_t[:, base:base + 512],
                                    pl[:, 0::2], pl[:, 1::2])
                                nc.vector.tensor_sub(
                                    wt_t[:, base + 512:base + 1024],
                                    pl[:, 0::2], pl[:, 1::2])
                        # --- D stage over the 4-pair batch ---
                        # wt_t block layout per pair (1024 cols):
                        #   {LL0 LL1 HL0 HL1 | LH0 LH1 HH0 HH1}
                        wt4 = wt_t.rearrange("p (b c) -> p b c", b=4)
                        for c0, s_sum, s_diff in (
                            (0, 0, 4),    # LL -> LLL / HLL
                            (256, 2, 6),  # HL -> LHL / HHL
                            (512, 1, 5),  # LH -> LLH / HLH
                            (768, 3, 7),  # HH -> LHH / HHH
                        ):
                            in0 = wt4[:, :, c0:c0 + 128]
                            in1 = wt4[:, :, c0 + 128:c0 + 256]
                            for s, op in ((s_sum, nc.vector.tensor_add),
                                          (s_diff, nc.vector.tensor_sub)):
                                ob = s * (KB * 128) + half * 512
                                out_ap = os_t[:, ob:ob + 512].rearrange(
                                    "p (b w) -> p b w", b=4)
                                op(out_ap, in0, in1)
                    # --- store this k-block: 8 subbands x [KB,128,128] ---
                    for s in range(8):
                        src_ap = os_t[:, s * KB * 128:(s + 1) * KB * 128]
                        nc.sync.dma_start(
                            o_d[s, g, :, kb * KB:(kb + 1) * KB, :],
                            src_ap.rearrange("p (k w) -> p k w", k=KB))
    nc.finalize()
    return nc


def _get_nc():
    if "nc" not in _CACHE:
        _CACHE["nc"] = _build_nc()
    return _CACHE["nc"]


def kernel(x, low_0, low_1, low_2, high_0, high_1, high_2):
    from concourse.bass_utils import run_bass_kernel_spmd

    x = np.ascontiguousarray(np.asarray(x, dtype=np.float32))
    ft = _build_filter_lhst()
    xs = x.reshape(N * C, D, H, W)
    in_maps = [
        {"x": np.ascontiguousarray(
            xs[c * G_PER_CORE:(c + 1) * G_PER_CORE]), "ft": ft}
        for c in range(NCORES)
    ]
    nc = _get_nc()
    res = run_bass_kernel_spmd(nc, in_maps, list(range(NCORES)))
    full = np.empty((8, N * C, KP, 128, 128), dtype=np.float32)
    for c in range(NCORES):
        full[:, c * G_PER_CORE:(c + 1) * G_PER_CORE] = \
            res.results[c]["out"].transpose(0, 1, 3, 2, 4)
    full = full.reshape(8, N, C, KP, 128, 128)
    return tuple(full[s] for s in range(8))
